# revision 5
# baseline (speedup 1.0000x reference)
"""BiARMA (2-layer ARMAConv GNN) Trainium2 kernel, 8-core SPMD — v2.

Uses A_hat@(xW) == (A_hat@x)@W to aggregate raw features first and apply
weights after aggregation:
  C1: gather x rows (full x replicated per core, plain node order), scale by
      per-edge norm (DVE), one-hot segment-sum matmul -> psum (A_hat x)^T,
      then per dest tile: pD = W1i^T agg + W1r^T xT (psum-accumulated root),
      h1 = relu(pD + b1).  No AllGather needed for layer 1.
  AG: h1 (bf16, 64-wide) AllGather in 2 chunks (aligned with C2 source
      blocks 0-1 / 2-3) overlapped under C1 tail.
  C2: gather h1 PAIR rows (two 64-col rows per 256B element), parity-split
      one-hots, same psum scheme; out = relu(W2i^T agg2 + W2r^T h1T + b2).
Output written transposed [64(40 used), 12544] per core; host re-permutes.
"""
import numpy as np
import ml_dtypes

N_CORES = 8
N_NODES = 100000
IN_CH, HID, CLS = 128, 64, 40
SHARD = 12500
SHARD_PAD = 12544            # 98 * 128
NT = SHARD_PAD // 128        # 98 dest tiles per core
G = 7                        # tiles per gather group
NG = NT // G                 # 14 groups
NBLK = 4                     # source blocks of 25088 rows (int16 indexable)
BLK = 25088
VPAD = N_CORES * SHARD_PAD   # 100352 table rows
NCHUNK = 2                   # AllGather chunks (7 groups each = 50176 rows)
GPCH = NG // NCHUNK          # groups per chunk
ROWS_CHUNK_CORE = GPCH * G * 128   # 6272 own rows per AG chunk

bf16 = ml_dtypes.bfloat16
SUBCALL = 1024   # dma_gather ucode cap on num_idxs (HW ring limit)

_PROG_CACHE = {}


def _pack_layout(c_tb1, c_tb2):
    """Column layout of the packed bf16 input tensor [128, total]."""
    TOT1 = NT * NBLK * c_tb1 * 128
    TOT2 = NT * NBLK * 2 * c_tb2 * 128
    widths = [
        ("xTown", SHARD_PAD), ("iota", 256),
        ("identb", 128), ("w1i", 64), ("w1r", 64), ("w2i", 64),
        ("w2r", 64), ("zeros", 128),
        ("colr1", TOT1 // 128), ("colr2", TOT2 // 128),
    ]
    lay = {}
    off = 0
    for name, w in widths:
        lay[name] = (off, w)
        off += w
    return lay, off


# ----------------------------------------------------------------------------
# host-side prep
# ----------------------------------------------------------------------------

def _pack_tiles(cnt):
    """Greedy balanced packing of SHARD_PAD dests into NT tiles of 128.

    cnt: [SHARD_PAD, NBLK] per-dest edge counts by C1 source block.
    Returns tile_of[d], slot_of[d] (slot in 0..127).
    """
    tot = cnt.sum(1)
    order = np.argsort(-tot, kind="stable")
    sums = np.zeros((NT, NBLK), np.int64)
    nd = np.zeros(NT, np.int64)
    tile_of = np.empty(SHARD_PAD, np.int32)
    slot_of = np.empty(SHARD_PAD, np.int32)
    BIG = 1 << 40
    for d in order:
        load = (sums + cnt[d]).max(axis=1) + (nd >= 128) * BIG
        t = int(np.argmin(load))
        tile_of[d] = t
        slot_of[d] = nd[t]
        nd[t] += 1
        sums[t] += cnt[d]
    return tile_of, slot_of


def _stream(et, eslot, srcrow, normv, c_tb):
    """Token stream for one phase on one core.

    et/eslot: dest tile and slot per edge; srcrow: table row per edge
    (phase-specific numbering); normv: per-edge norm.
    Layout: for g(NG): for b(NBLK): tiles g*G..g*G+G-1, c_tb*128 slots each.
    Returns (idx128 int16 [128, TOT//16], colr bf16 [128, TOT//128],
             norm bf16 [128, TOT//128]).
    """
    TOT = NT * NBLK * c_tb * 128
    eb = srcrow // BLK
    g = et // G
    base = ((g * NBLK + eb) * G + (et - g * G)) * (c_tb * 128)
    key = et * NBLK + eb
    order = np.argsort(key, kind="stable")
    ks = key[order]
    rank = np.arange(len(ks)) - np.searchsorted(ks, ks)
    pos = base[order] + rank
    tok_src = np.zeros(TOT, np.int16)
    tok_colr = np.full(TOT, 300.0, np.float32)
    tok_norm = np.zeros(TOT, np.float32)
    tok_src[pos] = (srcrow[order] % BLK).astype(np.int16)
    tok_colr[pos] = eslot[order].astype(np.float32)
    tok_norm[pos] = normv[order]
    idx16 = tok_src.reshape(TOT // 16, 16).T.copy()
    colr = tok_colr.reshape(TOT // 128, 128).T.astype(bf16).copy()
    norm = tok_norm.reshape(TOT // 128, 128).T.astype(bf16).copy()
    return idx16, colr, norm


def _stream2(et, eslot, srcrow, c_tb):
    """C2 variant: srcrow is an h1d-table row; gather PAIR index; tokens are
    separated into even-parity then odd-parity chunks within each (tile,
    block) run, so one-hots stay 128-wide and the matmul picks the 64-col
    half of the gathered pair by chunk parity."""
    TOT = NT * NBLK * 2 * c_tb * 128
    eb = srcrow // BLK
    par = srcrow % 2
    g = et // G
    base = (((g * NBLK + eb) * G + (et - g * G)) * 2 + par) * (c_tb * 128)
    key = (et * NBLK + eb) * 2 + par
    order = np.argsort(key, kind="stable")
    ks = key[order]
    rank = np.arange(len(ks)) - np.searchsorted(ks, ks)
    pos = base[order] + rank
    tok_src = np.zeros(TOT, np.int16)
    tok_colr = np.full(TOT, 300.0, np.float32)
    rel = srcrow[order] % BLK
    tok_src[pos] = (rel // 2).astype(np.int16)
    tok_colr[pos] = eslot[order].astype(np.float32)
    idx16 = tok_src.reshape(TOT // 16, 16).T.copy()
    colr = tok_colr.reshape(TOT // 128, 128).T.astype(bf16).copy()
    return idx16, colr


def _prep(edge_index):
    row = np.asarray(edge_index[0]).astype(np.int64)
    col = np.asarray(edge_index[1]).astype(np.int64)
    deg = np.bincount(col, minlength=N_NODES).astype(np.float64)
    dinv = np.where(deg > 0, 1.0 / np.sqrt(np.maximum(deg, 1e-12)), 0.0).astype(
        np.float32
    )
    dst_core = col // SHARD

    packs, edges = [], []
    for k in range(N_CORES):
        m = dst_core == k
        er = row[m]
        ec = col[m] - k * SHARD
        cnt = np.zeros((SHARD_PAD, NBLK), np.int64)
        np.add.at(cnt, (ec, er // BLK), 1)
        tile_of, slot_of = _pack_tiles(cnt)
        packs.append((tile_of, slot_of))
        edges.append((er, ec))

    # h1d-table row per global node: core-major (AllGather concat order)
    h1drow = np.empty(N_NODES, np.int64)
    perms = []
    for k in range(N_CORES):
        tile_of, slot_of = packs[k]
        d = np.arange(SHARD)
        t = tile_of[:SHARD].astype(np.int64)
        s = slot_of[:SHARD].astype(np.int64)
        h1drow[k * SHARD + d] = k * SHARD_PAD + t * 128 + s
        perm = np.full(SHARD_PAD, -1, np.int64)
        slot_idx = tile_of * 128 + slot_of
        dl = np.arange(SHARD_PAD)
        node = k * SHARD + dl
        valid = dl < SHARD
        perm[slot_idx[valid]] = node[valid]
        perms.append(perm)

    # per-core c_tb (max chunk count over (tile, blk)), shared across cores
    c_tb1 = c_tb2 = 1
    keyed = []
    for k in range(N_CORES):
        er, ec = edges[k]
        tile_of, slot_of = packs[k]
        et = tile_of[ec].astype(np.int64)
        cnt1 = np.zeros((NT, NBLK), np.int64)
        np.add.at(cnt1, (et, h1drow[er] // BLK), 1)
        cnt2 = np.zeros((NT, NBLK, 2), np.int64)
        np.add.at(cnt2, (et, h1drow[er] // BLK, h1drow[er] % 2), 1)
        c_tb1 = max(c_tb1, int(np.ceil(cnt1.max() / 128)))
        c_tb2 = max(c_tb2, int(np.ceil(cnt2.max() / 128)))
        keyed.append((er, ec, et, slot_of[ec]))

    streams = []
    for k in range(N_CORES):
        er, ec, et, eslot = keyed[k]
        normv = np.zeros(len(er), np.float32)  # unused (dinv folded)
        idx1, colr1, _ = _stream(et, eslot, h1drow[er], normv, c_tb1)
        idx2, colr2 = _stream2(et, eslot, h1drow[er], c_tb2)
        # dinv of own dests in slot order (0 for pad slots)
        perm = perms[k]
        dinv_slot = np.zeros(SHARD_PAD, np.float32)
        v = perm >= 0
        dinv_slot[v] = dinv[perm[v]]
        streams.append(
            dict(idx1=idx1, colr1=colr1, idx2=idx2, colr2=colr2,
                 perm=perm, dinv_slot=dinv_slot)
        )
    return streams, c_tb1, c_tb2, dinv


# ----------------------------------------------------------------------------
# device program
# ----------------------------------------------------------------------------

def _build_program(c_tb1, c_tb2, debug=False):
    import concourse.bacc as bacc
    import concourse.mybir as mybir
    import concourse.tile as tile
    from concourse import library_config

    f32 = mybir.dt.float32
    b16 = mybir.dt.bfloat16
    i16 = mybir.dt.int16
    Copy = mybir.ActivationFunctionType.Copy
    Relu = mybir.ActivationFunctionType.Relu
    ISEQ = mybir.AluOpType.is_equal
    MULT = mybir.AluOpType.mult

    TOT1 = NT * NBLK * c_tb1 * 128
    TOT2 = NT * NBLK * 2 * c_tb2 * 128
    N1 = G * c_tb1 * 128        # tokens per C1 (g,b) call
    N2 = G * 2 * c_tb2 * 128    # tokens per C2 (g,b) call (even+odd chunks)

    nc = bacc.Bacc("TRN2", target_bir_lowering=False, debug=False,
                   num_devices=N_CORES)

    lay, P16W = _pack_layout(c_tb1, c_tb2)
    t_pack16 = nc.dram_tensor("pack16", [128, P16W], b16, kind="ExternalInput")
    t_packi = nc.dram_tensor("packi", [16, (TOT1 + TOT2) // 16], i16,
                             kind="ExternalInput")
    t_packf = nc.dram_tensor("packf", [128, NT + 2], f32, kind="ExternalInput")
    t_dinvrow = nc.dram_tensor("dinvrow", [1, SHARD_PAD], b16,
                               kind="ExternalInput")
    t_out = nc.dram_tensor("outT", [64, SHARD_PAD], b16, kind="ExternalOutput")
    if debug:
        t_dbg_agg = nc.dram_tensor("dbg_agg", [128, SHARD_PAD], b16,
                                   kind="ExternalOutput")
        t_dbg_h1T = nc.dram_tensor("dbg_h1T", [64, SHARD_PAD], b16,
                                   kind="ExternalOutput")
        t_dbg_h1d = nc.dram_tensor("dbg_h1d", [SHARD_PAD, 64], b16,
                                   kind="ExternalOutput")
        t_dbg_agg2 = nc.dram_tensor("dbg_agg2", [64, SHARD_PAD], b16,
                                    kind="ExternalOutput")

    with tile.TileContext(nc) as tc:
        with (
            tc.tile_pool(name="cst", bufs=1) as cst,
            tc.tile_pool(name="mp", bufs=4) as mp,
            tc.tile_pool(name="ohp", bufs=4) as ohp,
            tc.tile_pool(name="aggp", bufs=3) as aggp,
            tc.tile_pool(name="h1tp", bufs=NT) as h1tp,
            tc.tile_pool(name="tbp", bufs=3) as tbp,
            tc.tile_pool(name="outp", bufs=3) as outp,
            tc.tile_pool(name="psC", bufs=2, space="PSUM") as psC,
            tc.tile_pool(name="psD", bufs=2, space="PSUM") as psD,
            tc.tile_pool(name="psT", bufs=2, space="PSUM") as psT,
            tc.tile_pool(name="dram", bufs=1, space="DRAM") as dram,
        ):
            nc.gpsimd.load_library(library_config.mlp)

            def load16(name, nrows=128):
                off, w = lay[name]
                s = cst.tile([nrows, w], b16, tag=f"{name}_s", name=f"{name}_s")
                nc.sync.dma_start(s[:], t_pack16[:nrows, off:off + w])
                return s

            iota_s = load16("iota")
            identb_s = load16("identb")
            w1i_s = load16("w1i")
            w1r_s = load16("w1r")
            w2i_s = load16("w2i", 64)
            w2r_s = load16("w2r", 64)
            zero_s = load16("zeros")
            xT_s = load16("xTown")
            dinvrow = cst.tile([1, SHARD_PAD], b16, tag="dinvrow",
                               name="dinvrow")
            nc.sync.dma_start(dinvrow[:], t_dinvrow[:])
            dinvb_s = cst.tile([128, SHARD_PAD], b16, tag="dinvb_s",
                               name="dinvb_s")
            nc.gpsimd.partition_broadcast(dinvb_s[:], dinvrow[:])
            colr1_s = load16("colr1")
            colr2_s = load16("colr2")
            b1_s = cst.tile([64, 1], f32, tag="b1_s", name="b1_s")
            nc.sync.dma_start(b1_s[:], t_packf[:64, NT:NT + 1])
            b2_s = cst.tile([64, 1], f32, tag="b2_s", name="b2_s")
            nc.sync.dma_start(b2_s[:], t_packf[:64, NT + 1:NT + 2])
            dinvT_s = cst.tile([128, NT], f32, tag="dinvT_s", name="dinvT_s")
            nc.sync.dma_start(dinvT_s[:], t_packf[:, 0:NT])
            idx1_s = cst.tile([128, TOT1 // 16], i16, tag="idx1_s", name="idx1_s")
            idx2_s = cst.tile([128, TOT2 // 16], i16, tag="idx2_s", name="idx2_s")
            for r in range(8):
                nc.sync.dma_start(idx1_s[r * 16:(r + 1) * 16, :],
                                  t_packi[:, 0:TOT1 // 16])
                nc.sync.dma_start(idx2_s[r * 16:(r + 1) * 16, :],
                                  t_packi[:, TOT1 // 16:(TOT1 + TOT2) // 16])

            h1d_own = dram.tile([SHARD_PAD, 64], b16)
            h1d_full = dram.tile([VPAD, 64], b16)
            xsh_own = dram.tile([SHARD_PAD, 128], b16)
            xtab_full = dram.tile([VPAD, 128], b16)

            h1T = []  # per dest tile [64, 128] bf16 (unscaled h1, feat-major)

            # ------- build dinv*x table shard (slot order) + AllGather -------
            for t in range(NT):
                pX = psT.tile([128, 128], b16, tag="psT",
                              padded_shape=[128, 1024])
                nc.tensor.transpose(out=pX[:], in_=xT_s[:, t * 128:(t + 1) * 128],
                                    identity=identb_s[:])
                xb = tbp.tile([128, 128], b16, tag="tbp")
                nc.vector.tensor_tensor(
                    out=xb[:], in0=pX[:],
                    in1=dinvT_s[:, t:t + 1].broadcast_to([128, 128]), op=MULT)
                nc.sync.dma_start(xsh_own[t * 128:(t + 1) * 128, :], xb[:])
            nc.gpsimd.collective_compute(
                "AllGather", mybir.AluOpType.bypass,
                replica_groups=[list(range(N_CORES))],
                ins=[xsh_own.opt()], outs=[xtab_full.opt()],
            )

            # ---------------- C1: edge aggregation + layer-1 node math -------
            for g in range(NG):
                nch1 = G * c_tb1
                pC = psC.tile([128, 1024], f32, tag="psC")
                # start=True resets the full PSUM bank, so zero-init each
                # bank once and accumulate with start=False everywhere
                for bk in range(2):
                    nc.tensor.matmul(out=pC[:, bk * 512:(bk + 1) * 512],
                                     lhsT=zero_s[:], rhs=colr1_s[:, 0:512],
                                     start=True, stop=False)
                for b in range(NBLK):
                    off = (g * NBLK + b) * N1
                    m = mp.tile([128, N1], b16, tag="mp")
                    done = 0
                    while done < N1:
                        step = min(SUBCALL, N1 - done)
                        nc.gpsimd.dma_gather(
                            out_ap=m[:, done:done + step].rearrange(
                                "p (c d) -> p c d", d=128),
                            in_ap=xtab_full[b * BLK:(b + 1) * BLK, :],
                            idxs_ap=idx1_s[:, (off + done) // 16:
                                           (off + done + step) // 16],
                            num_idxs=step,
                            num_idxs_reg=step,
                            elem_size=128,
                        )
                        done += step
                    jg = off // 128
                    oh = ohp.tile([128, nch1 * 128], b16, tag="ohp")
                    nc.vector.tensor_tensor(
                        out=oh[:].rearrange("p (c d) -> p c d", d=128),
                        in0=iota_s[:, 0:128].unsqueeze(1).broadcast_to(
                            [128, nch1, 128]),
                        in1=colr1_s[:, jg:jg + nch1].unsqueeze(2).broadcast_to(
                            [128, nch1, 128]),
                        op=ISEQ,
                    )
                    for ti in range(G):
                        last_of_bank = min(G - 1, (ti // 4) * 4 + 3)
                        for c in range(c_tb1):
                            j = (ti * c_tb1 + c) * 128
                            nc.tensor.matmul(
                                out=pC[:, ti * 128:(ti + 1) * 128],
                                lhsT=m[:, j:j + 128],
                                rhs=oh[:, j:j + 128],
                                start=False,
                                stop=(b == NBLK - 1 and c == c_tb1 - 1
                                      and ti == last_of_bank),
                            )
                # readout: per tile pD = W1i^T (dinv_dst*agg) + W1r^T xT ;
                # h1 = relu(+b1)
                for ti in range(G):
                    t = g * G + ti
                    agg = aggp.tile([128, 128], b16, tag="aggp")
                    nc.vector.tensor_tensor(
                        out=agg[:], in0=pC[:, ti * 128:(ti + 1) * 128],
                        in1=dinvb_s[:, t * 128:(t + 1) * 128], op=MULT)
                    if debug:
                        nc.sync.dma_start(
                            t_dbg_agg[:, t * 128:(t + 1) * 128], agg[:])
                    pD = psD.tile([64, 128], f32, tag="psD",
                                  padded_shape=[128, 512])
                    nc.tensor.matmul(out=pD[:], lhsT=w1i_s[:], rhs=agg[:],
                                     start=True, stop=False)
                    nc.tensor.matmul(out=pD[:], lhsT=w1r_s[:],
                                     rhs=xT_s[:, t * 128:(t + 1) * 128],
                                     start=False, stop=True)
                    h1t = h1tp.tile([64, 128], b16, tag="h1tp")
                    nc.scalar.activation(h1t[:], pD[:], Relu, bias=b1_s[:])
                    h1T.append(h1t)
                    # node-major table row write, scaled by dinv_src
                    pT = psT.tile([128, 64], b16, tag="psT",
                                  padded_shape=[128, 1024])
                    nc.tensor.transpose(out=pT[:], in_=h1t[:],
                                        identity=identb_s[:64, :64])
                    tb = tbp.tile([128, 64], b16, tag="tbp")
                    nc.vector.tensor_tensor(
                        out=tb[:], in0=pT[:],
                        in1=dinvT_s[:, t:t + 1].broadcast_to([128, 64]),
                        op=MULT)
                    nc.sync.dma_start(h1d_own[t * 128:(t + 1) * 128, :], tb[:])
                    if debug:
                        nc.sync.dma_start(
                            t_dbg_h1T[:, t * 128:(t + 1) * 128], h1t[:])
                        nc.sync.dma_start(
                            t_dbg_h1d[t * 128:(t + 1) * 128, :], tb[:])
            # ---------------- AllGather of the layer-1 table -----------------
            nc.gpsimd.collective_compute(
                "AllGather", mybir.AluOpType.bypass,
                replica_groups=[list(range(N_CORES))],
                ins=[h1d_own.opt()],
                outs=[h1d_full.opt()],
            )

            # ---------------- C2: layer-2 aggregation + output ---------------
            for g in range(NG):
                nch2 = G * 2 * c_tb2
                pC = psC.tile([128, 1024], f32, tag="psC")
                for bk in range(2):
                    nc.tensor.matmul(out=pC[:, bk * 512:(bk + 1) * 512],
                                     lhsT=zero_s[:], rhs=colr1_s[:, 0:512],
                                     start=True, stop=False)
                for b in range(NBLK):
                    off = (g * NBLK + b) * N2
                    m = mp.tile([128, N2], b16, tag="mp")
                    tab = h1d_full[b * BLK:(b + 1) * BLK, :]
                    done = 0
                    while done < N2:
                        step = min(SUBCALL, N2 - done)
                        nc.gpsimd.dma_gather(
                            out_ap=m[:, done:done + step].rearrange(
                                "p (c d) -> p c d", d=128),
                            in_ap=tab.rearrange("(p two) f -> p (two f)", two=2),
                            idxs_ap=idx2_s[:, (off + done) // 16:
                                           (off + done + step) // 16],
                            num_idxs=step,
                            num_idxs_reg=step,
                            elem_size=128,
                        )
                        done += step
                    jg = off // 128
                    oh = ohp.tile([128, nch2 * 128], b16, tag="ohp")
                    nc.vector.tensor_tensor(
                        out=oh[:].rearrange("p (c d) -> p c d", d=128),
                        in0=iota_s[:, 0:128].unsqueeze(1).broadcast_to(
                            [128, nch2, 128]),
                        in1=colr2_s[:, jg:jg + nch2].unsqueeze(2).broadcast_to(
                            [128, nch2, 128]),
                        op=ISEQ,
                    )
                    for ti in range(G):
                        last_of_bank = min(G - 1, (ti // 4) * 4 + 3)
                        for par in range(2):
                            for c in range(c_tb2):
                                j = (ti * 2 + par) * c_tb2 + c
                                nc.tensor.matmul(
                                    out=pC[:64, ti * 128:(ti + 1) * 128],
                                    lhsT=m[:, j * 128 + par * 64:
                                           j * 128 + par * 64 + 64],
                                    rhs=oh[:, j * 128:(j + 1) * 128],
                                    start=False,
                                    stop=(b == NBLK - 1 and par == 1
                                          and c == c_tb2 - 1
                                          and ti == last_of_bank),
                                )
                for ti in range(G):
                    t = g * G + ti
                    agg = aggp.tile([64, 128], b16, tag="aggp")
                    nc.vector.tensor_tensor(
                        out=agg[:], in0=pC[:64, ti * 128:(ti + 1) * 128],
                        in1=dinvb_s[:64, t * 128:(t + 1) * 128], op=MULT)
                    if debug:
                        nc.sync.dma_start(
                            t_dbg_agg2[:, t * 128:(t + 1) * 128], agg[:])
                    pO = psD.tile([64, 128], f32, tag="psD",
                                  padded_shape=[128, 512])
                    nc.tensor.matmul(out=pO[:], lhsT=w2i_s[:], rhs=agg[:],
                                     start=True, stop=False)
                    nc.tensor.matmul(out=pO[:], lhsT=w2r_s[:], rhs=h1T[t][:],
                                     start=False, stop=True)
                    ot = outp.tile([64, 128], b16, tag="outp")
                    nc.scalar.activation(ot[:], pO[:], Relu, bias=b2_s[:])
                    nc.sync.dma_start(t_out[:, t * 128:(t + 1) * 128], ot[:])

    nc.compile()
    return nc


# ----------------------------------------------------------------------------
# entry point
# ----------------------------------------------------------------------------

_LAST_RESULTS = None


def _in_maps(x, w1_init, w1_root, b1, w2_init, w2_root, b2, streams, dinv):
    x = np.asarray(x, np.float32)
    iota = np.broadcast_to(np.arange(256, dtype=np.float32), (128, 256)).astype(bf16)
    identb = np.eye(128, dtype=np.float32).astype(bf16)
    w1i = np.asarray(w1_init, np.float32).astype(bf16)            # [128, 64]
    w1r = np.asarray(w1_root, np.float32).astype(bf16)            # [128, 64]
    w2i = np.zeros((64, 64), np.float32)
    w2i[:, :CLS] = np.asarray(w2_init)
    w2r = np.zeros((64, 64), np.float32)
    w2r[:, :CLS] = np.asarray(w2_root)
    b1c = np.asarray(b1, np.float32).reshape(64, 1).copy()
    b2c = np.zeros((64, 1), np.float32)
    b2c[:CLS, 0] = np.asarray(b2)

    c_tb1 = streams[0]["idx1"].shape[1] * 16 // (NT * NBLK * 128)
    c_tb2 = streams[0]["idx2"].shape[1] * 16 // (NT * NBLK * 2 * 128)
    lay, P16W = _pack_layout(c_tb1, c_tb2)

    def put(pack, name, arr):
        off, w = lay[name]
        arr = np.asarray(arr, dtype=np.float32).astype(bf16)
        pack[:arr.shape[0], off:off + arr.shape[1]] = arr

    maps = []
    for k in range(N_CORES):
        s = streams[k]
        perm = s["perm"]
        v = perm >= 0
        xT = np.zeros((128, SHARD_PAD), np.float32)
        xT[:, v] = x[perm[v]].T
        dinvT = s["dinv_slot"].reshape(NT, 128).T.copy()   # [128 slot, NT]
        pack16 = np.zeros((128, P16W), bf16)
        put(pack16, "xTown", xT)
        put(pack16, "iota", np.asarray(iota))
        put(pack16, "identb", identb)
        put(pack16, "w1i", w1i)
        put(pack16, "w1r", w1r)
        put(pack16, "w2i", w2i)
        put(pack16, "w2r", w2r)
        pack16[:, lay["colr1"][0]:lay["colr1"][0] + lay["colr1"][1]] = s["colr1"]
        pack16[:, lay["colr2"][0]:lay["colr2"][0] + lay["colr2"][1]] = s["colr2"]
        packi = np.concatenate([s["idx1"], s["idx2"]], axis=1)
        packf = np.zeros((128, NT + 2), np.float32)
        packf[:, :NT] = dinvT
        packf[:64, NT] = b1c[:, 0]
        packf[:64, NT + 1] = b2c[:, 0]
        maps.append(dict(pack16=pack16, packi=packi, packf=packf,
                         dinvrow=s["dinv_slot"].reshape(1, SHARD_PAD)
                         .astype(bf16)))
    return maps


def kernel(x, edge_index, w1_init, w1_root, b1, w2_init, w2_root, b2, **kw):
    global _LAST_RESULTS
    from concourse.bass_utils import run_bass_kernel_spmd

    streams, c_tb1, c_tb2, dinv = _prep(np.asarray(edge_index))
    key = (c_tb1, c_tb2)
    if key not in _PROG_CACHE:
        _PROG_CACHE[key] = _build_program(c_tb1, c_tb2)
    nc = _PROG_CACHE[key]

    maps = _in_maps(x, w1_init, w1_root, b1, w2_init, w2_root, b2, streams,
                    dinv)
    res = run_bass_kernel_spmd(nc, maps, core_ids=list(range(N_CORES)))
    _LAST_RESULTS = res

    out = np.zeros((N_NODES, CLS), np.float32)
    for k in range(N_CORES):
        o = res.results[k]["outT"]          # [64, SHARD_PAD]
        perm = streams[k]["perm"]
        v = perm >= 0
        out[perm[v]] = np.asarray(o[:CLS, v].T, np.float32)
    return out


# revision 6
# speedup vs baseline: 1.0840x; 1.0840x over previous
"""BiARMA (2-layer ARMAConv GNN) Trainium2 kernel, 8-core SPMD — v2.

Uses A_hat@(xW) == (A_hat@x)@W to aggregate raw features first and apply
weights after aggregation:
  C1: gather x rows (full x replicated per core, plain node order), scale by
      per-edge norm (DVE), one-hot segment-sum matmul -> psum (A_hat x)^T,
      then per dest tile: pD = W1i^T agg + W1r^T xT (psum-accumulated root),
      h1 = relu(pD + b1).  No AllGather needed for layer 1.
  AG: h1 (bf16, 64-wide) AllGather in 2 chunks (aligned with C2 source
      blocks 0-1 / 2-3) overlapped under C1 tail.
  C2: gather h1 PAIR rows (two 64-col rows per 256B element), parity-split
      one-hots, same psum scheme; out = relu(W2i^T agg2 + W2r^T h1T + b2).
Output written transposed [64(40 used), 12544] per core; host re-permutes.
"""
import numpy as np
import ml_dtypes

N_CORES = 8
N_NODES = 100000
IN_CH, HID, CLS = 128, 64, 40
SHARD = 12500
SHARD_PAD = 12544            # 98 * 128
NT = SHARD_PAD // 128        # 98 dest tiles per core
G = 7                        # tiles per gather group
NG = NT // G                 # 14 groups
NBLK = 4                     # source blocks of 25088 rows (int16 indexable)
BLK = 25088
VPAD = N_CORES * SHARD_PAD   # 100352 table rows
NCHUNK = 2                   # AllGather chunks (7 groups each = 50176 rows)
GPCH = NG // NCHUNK          # groups per chunk
ROWS_CHUNK_CORE = GPCH * G * 128   # 6272 own rows per AG chunk

bf16 = ml_dtypes.bfloat16
SUBCALL = 1024   # dma_gather ucode cap on num_idxs (HW ring limit)

_PROG_CACHE = {}


def _pack_layout(c_tb1, c_tb2):
    """Column layout of the packed bf16 input tensor [128, total]."""
    TOT1 = NT * NBLK * c_tb1 * 128
    TOT2 = NT * NBLK * 2 * c_tb2 * 128
    widths = [
        ("xTown", SHARD_PAD), ("dinvb", SHARD_PAD), ("iota", 256),
        ("identb", 128), ("w1i", 64), ("w1r", 64), ("w2i", 64),
        ("w2r", 64), ("zeros", 128),
        ("colr1", TOT1 // 128), ("colr2", TOT2 // 128),
    ]
    lay = {}
    off = 0
    for name, w in widths:
        lay[name] = (off, w)
        off += w
    return lay, off


# ----------------------------------------------------------------------------
# host-side prep
# ----------------------------------------------------------------------------

def _pack_tiles(cnt):
    """Greedy balanced packing of SHARD_PAD dests into NT tiles of 128.

    cnt: [SHARD_PAD, NBLK] per-dest edge counts by C1 source block.
    Returns tile_of[d], slot_of[d] (slot in 0..127).
    """
    tot = cnt.sum(1)
    order = np.argsort(-tot, kind="stable")
    sums = np.zeros((NT, NBLK), np.int64)
    nd = np.zeros(NT, np.int64)
    tile_of = np.empty(SHARD_PAD, np.int32)
    slot_of = np.empty(SHARD_PAD, np.int32)
    BIG = 1 << 40
    for d in order:
        load = (sums + cnt[d]).max(axis=1) + (nd >= 128) * BIG
        t = int(np.argmin(load))
        tile_of[d] = t
        slot_of[d] = nd[t]
        nd[t] += 1
        sums[t] += cnt[d]
    return tile_of, slot_of


def _stream(et, eslot, srcrow, normv, c_tb):
    """Token stream for one phase on one core.

    et/eslot: dest tile and slot per edge; srcrow: table row per edge
    (phase-specific numbering); normv: per-edge norm.
    Layout: for g(NG): for b(NBLK): tiles g*G..g*G+G-1, c_tb*128 slots each.
    Returns (idx128 int16 [128, TOT//16], colr bf16 [128, TOT//128],
             norm bf16 [128, TOT//128]).
    """
    TOT = NT * NBLK * c_tb * 128
    eb = srcrow // BLK
    g = et // G
    base = ((g * NBLK + eb) * G + (et - g * G)) * (c_tb * 128)
    key = et * NBLK + eb
    order = np.argsort(key, kind="stable")
    ks = key[order]
    rank = np.arange(len(ks)) - np.searchsorted(ks, ks)
    pos = base[order] + rank
    tok_src = np.zeros(TOT, np.int16)
    tok_colr = np.full(TOT, 300.0, np.float32)
    tok_norm = np.zeros(TOT, np.float32)
    tok_src[pos] = (srcrow[order] % BLK).astype(np.int16)
    tok_colr[pos] = eslot[order].astype(np.float32)
    tok_norm[pos] = normv[order]
    idx16 = tok_src.reshape(TOT // 16, 16).T
    idx128 = np.tile(idx16, (8, 1)).copy()
    colr = tok_colr.reshape(TOT // 128, 128).T.astype(bf16).copy()
    norm = tok_norm.reshape(TOT // 128, 128).T.astype(bf16).copy()
    return idx128, colr, norm


def _stream2(et, eslot, srcrow, c_tb):
    """C2 variant: srcrow is an h1d-table row; gather PAIR index; tokens are
    separated into even-parity then odd-parity chunks within each (tile,
    block) run, so one-hots stay 128-wide and the matmul picks the 64-col
    half of the gathered pair by chunk parity."""
    TOT = NT * NBLK * 2 * c_tb * 128
    eb = srcrow // BLK
    par = srcrow % 2
    g = et // G
    base = (((g * NBLK + eb) * G + (et - g * G)) * 2 + par) * (c_tb * 128)
    key = (et * NBLK + eb) * 2 + par
    order = np.argsort(key, kind="stable")
    ks = key[order]
    rank = np.arange(len(ks)) - np.searchsorted(ks, ks)
    pos = base[order] + rank
    tok_src = np.zeros(TOT, np.int16)
    tok_colr = np.full(TOT, 300.0, np.float32)
    rel = srcrow[order] % BLK
    tok_src[pos] = (rel // 2).astype(np.int16)
    tok_colr[pos] = eslot[order].astype(np.float32)
    idx16 = tok_src.reshape(TOT // 16, 16).T
    idx128 = np.tile(idx16, (8, 1)).copy()
    colr = tok_colr.reshape(TOT // 128, 128).T.astype(bf16).copy()
    return idx128, colr


def _prep(edge_index):
    row = np.asarray(edge_index[0]).astype(np.int64)
    col = np.asarray(edge_index[1]).astype(np.int64)
    deg = np.bincount(col, minlength=N_NODES).astype(np.float64)
    dinv = np.where(deg > 0, 1.0 / np.sqrt(np.maximum(deg, 1e-12)), 0.0).astype(
        np.float32
    )
    dst_core = col // SHARD

    packs, edges = [], []
    for k in range(N_CORES):
        m = dst_core == k
        er = row[m]
        ec = col[m] - k * SHARD
        cnt = np.zeros((SHARD_PAD, NBLK), np.int64)
        np.add.at(cnt, (ec, er // BLK), 1)
        tile_of, slot_of = _pack_tiles(cnt)
        packs.append((tile_of, slot_of))
        edges.append((er, ec))

    # h1d-table row per global node: core-major (AllGather concat order)
    h1drow = np.empty(N_NODES, np.int64)
    perms = []
    for k in range(N_CORES):
        tile_of, slot_of = packs[k]
        d = np.arange(SHARD)
        t = tile_of[:SHARD].astype(np.int64)
        s = slot_of[:SHARD].astype(np.int64)
        h1drow[k * SHARD + d] = k * SHARD_PAD + t * 128 + s
        perm = np.full(SHARD_PAD, -1, np.int64)
        slot_idx = tile_of * 128 + slot_of
        dl = np.arange(SHARD_PAD)
        node = k * SHARD + dl
        valid = dl < SHARD
        perm[slot_idx[valid]] = node[valid]
        perms.append(perm)

    # per-core c_tb (max chunk count over (tile, blk)), shared across cores
    c_tb1 = c_tb2 = 1
    keyed = []
    for k in range(N_CORES):
        er, ec = edges[k]
        tile_of, slot_of = packs[k]
        et = tile_of[ec].astype(np.int64)
        cnt1 = np.zeros((NT, NBLK), np.int64)
        np.add.at(cnt1, (et, er // BLK), 1)
        cnt2 = np.zeros((NT, NBLK, 2), np.int64)
        np.add.at(cnt2, (et, h1drow[er] // BLK, h1drow[er] % 2), 1)
        c_tb1 = max(c_tb1, int(np.ceil(cnt1.max() / 128)))
        c_tb2 = max(c_tb2, int(np.ceil(cnt2.max() / 128)))
        keyed.append((er, ec, et, slot_of[ec]))

    streams = []
    for k in range(N_CORES):
        er, ec, et, eslot = keyed[k]
        normv = np.zeros(len(er), np.float32)  # unused (dinv folded)
        idx1, colr1, _ = _stream(et, eslot, er, normv, c_tb1)
        idx2, colr2 = _stream2(et, eslot, h1drow[er], c_tb2)
        # dinv of own dests in slot order (0 for pad slots)
        perm = perms[k]
        dinv_slot = np.zeros(SHARD_PAD, np.float32)
        v = perm >= 0
        dinv_slot[v] = dinv[perm[v]]
        streams.append(
            dict(idx1=idx1, colr1=colr1, idx2=idx2, colr2=colr2,
                 perm=perm, dinv_slot=dinv_slot)
        )
    return streams, c_tb1, c_tb2, dinv


# ----------------------------------------------------------------------------
# device program
# ----------------------------------------------------------------------------

def _build_program(c_tb1, c_tb2, debug=False):
    import concourse.bacc as bacc
    import concourse.mybir as mybir
    import concourse.tile as tile
    from concourse import library_config

    f32 = mybir.dt.float32
    b16 = mybir.dt.bfloat16
    i16 = mybir.dt.int16
    Copy = mybir.ActivationFunctionType.Copy
    Relu = mybir.ActivationFunctionType.Relu
    ISEQ = mybir.AluOpType.is_equal
    MULT = mybir.AluOpType.mult

    TOT1 = NT * NBLK * c_tb1 * 128
    TOT2 = NT * NBLK * 2 * c_tb2 * 128
    N1 = G * c_tb1 * 128        # tokens per C1 (g,b) call
    N2 = G * 2 * c_tb2 * 128    # tokens per C2 (g,b) call (even+odd chunks)

    nc = bacc.Bacc("TRN2", target_bir_lowering=False, debug=False,
                   num_devices=N_CORES)

    lay, P16W = _pack_layout(c_tb1, c_tb2)
    t_xtab = nc.dram_tensor("xtab", [VPAD, 128], b16, kind="ExternalInput")
    t_pack16 = nc.dram_tensor("pack16", [128, P16W], b16, kind="ExternalInput")
    t_packi = nc.dram_tensor("packi", [128, (TOT1 + TOT2) // 16], i16,
                             kind="ExternalInput")
    t_packf = nc.dram_tensor("packf", [128, NT + 2], f32, kind="ExternalInput")
    t_out = nc.dram_tensor("outT", [64, SHARD_PAD], f32, kind="ExternalOutput")
    if debug:
        t_dbg_agg = nc.dram_tensor("dbg_agg", [128, SHARD_PAD], b16,
                                   kind="ExternalOutput")
        t_dbg_h1T = nc.dram_tensor("dbg_h1T", [64, SHARD_PAD], b16,
                                   kind="ExternalOutput")
        t_dbg_h1d = nc.dram_tensor("dbg_h1d", [SHARD_PAD, 64], b16,
                                   kind="ExternalOutput")
        t_dbg_agg2 = nc.dram_tensor("dbg_agg2", [64, SHARD_PAD], b16,
                                    kind="ExternalOutput")

    with tile.TileContext(nc) as tc:
        with (
            tc.tile_pool(name="cst", bufs=1) as cst,
            tc.tile_pool(name="mp", bufs=4) as mp,
            tc.tile_pool(name="ohp", bufs=4) as ohp,
            tc.tile_pool(name="aggp", bufs=3) as aggp,
            tc.tile_pool(name="h1tp", bufs=NT) as h1tp,
            tc.tile_pool(name="tbp", bufs=3) as tbp,
            tc.tile_pool(name="outp", bufs=3) as outp,
            tc.tile_pool(name="psC", bufs=2, space="PSUM") as psC,
            tc.tile_pool(name="psD", bufs=2, space="PSUM") as psD,
            tc.tile_pool(name="psT", bufs=2, space="PSUM") as psT,
            tc.tile_pool(name="dram", bufs=1, space="DRAM") as dram,
        ):
            nc.gpsimd.load_library(library_config.mlp)

            def load16(name, nrows=128):
                off, w = lay[name]
                s = cst.tile([nrows, w], b16, tag=f"{name}_s", name=f"{name}_s")
                nc.sync.dma_start(s[:], t_pack16[:nrows, off:off + w])
                return s

            iota_s = load16("iota")
            identb_s = load16("identb")
            w1i_s = load16("w1i")
            w1r_s = load16("w1r")
            w2i_s = load16("w2i", 64)
            w2r_s = load16("w2r", 64)
            zero_s = load16("zeros")
            xT_s = load16("xTown")
            dinvb_s = load16("dinvb")
            colr1_s = load16("colr1")
            colr2_s = load16("colr2")
            b1_s = cst.tile([64, 1], f32, tag="b1_s", name="b1_s")
            nc.sync.dma_start(b1_s[:], t_packf[:64, NT:NT + 1])
            b2_s = cst.tile([64, 1], f32, tag="b2_s", name="b2_s")
            nc.sync.dma_start(b2_s[:], t_packf[:64, NT + 1:NT + 2])
            dinvT_s = cst.tile([128, NT], f32, tag="dinvT_s", name="dinvT_s")
            nc.sync.dma_start(dinvT_s[:], t_packf[:, 0:NT])
            idx1_s = cst.tile([128, TOT1 // 16], i16, tag="idx1_s", name="idx1_s")
            nc.sync.dma_start(idx1_s[:], t_packi[:, 0:TOT1 // 16])
            idx2_s = cst.tile([128, TOT2 // 16], i16, tag="idx2_s", name="idx2_s")
            nc.sync.dma_start(idx2_s[:], t_packi[:, TOT1 // 16:(TOT1 + TOT2) // 16])

            h1d_own = dram.tile([SHARD_PAD, 64], b16)
            h1d_full = dram.tile([VPAD, 64], b16)

            h1T = []  # per dest tile [64, 128] bf16 (unscaled h1, feat-major)

            # ---------------- C1: edge aggregation + layer-1 node math -------
            for g in range(NG):
                nch1 = G * c_tb1
                pC = psC.tile([128, 1024], f32, tag="psC")
                # start=True resets the full PSUM bank, so zero-init each
                # bank once and accumulate with start=False everywhere
                for bk in range(2):
                    nc.tensor.matmul(out=pC[:, bk * 512:(bk + 1) * 512],
                                     lhsT=zero_s[:], rhs=colr1_s[:, 0:512],
                                     start=True, stop=False)
                for b in range(NBLK):
                    off = (g * NBLK + b) * N1
                    m = mp.tile([128, N1], b16, tag="mp")
                    done = 0
                    while done < N1:
                        step = min(SUBCALL, N1 - done)
                        nc.gpsimd.dma_gather(
                            out_ap=m[:, done:done + step].rearrange(
                                "p (c d) -> p c d", d=128),
                            in_ap=t_xtab[b * BLK:(b + 1) * BLK, :],
                            idxs_ap=idx1_s[:, (off + done) // 16:
                                           (off + done + step) // 16],
                            num_idxs=step,
                            num_idxs_reg=step,
                            elem_size=128,
                        )
                        done += step
                    jg = off // 128
                    oh = ohp.tile([128, nch1 * 128], b16, tag="ohp")
                    nc.vector.tensor_tensor(
                        out=oh[:].rearrange("p (c d) -> p c d", d=128),
                        in0=iota_s[:, 0:128].unsqueeze(1).broadcast_to(
                            [128, nch1, 128]),
                        in1=colr1_s[:, jg:jg + nch1].unsqueeze(2).broadcast_to(
                            [128, nch1, 128]),
                        op=ISEQ,
                    )
                    for ti in range(G):
                        last_of_bank = min(G - 1, (ti // 4) * 4 + 3)
                        for c in range(c_tb1):
                            j = (ti * c_tb1 + c) * 128
                            nc.tensor.matmul(
                                out=pC[:, ti * 128:(ti + 1) * 128],
                                lhsT=m[:, j:j + 128],
                                rhs=oh[:, j:j + 128],
                                start=False,
                                stop=(b == NBLK - 1 and c == c_tb1 - 1
                                      and ti == last_of_bank),
                            )
                # readout: per tile pD = W1i^T (dinv_dst*agg) + W1r^T xT ;
                # h1 = relu(+b1)
                for ti in range(G):
                    t = g * G + ti
                    agg = aggp.tile([128, 128], b16, tag="aggp")
                    nc.vector.tensor_tensor(
                        out=agg[:], in0=pC[:, ti * 128:(ti + 1) * 128],
                        in1=dinvb_s[:, t * 128:(t + 1) * 128], op=MULT)
                    if debug:
                        nc.sync.dma_start(
                            t_dbg_agg[:, t * 128:(t + 1) * 128], agg[:])
                    pD = psD.tile([64, 128], f32, tag="psD",
                                  padded_shape=[128, 512])
                    nc.tensor.matmul(out=pD[:], lhsT=w1i_s[:], rhs=agg[:],
                                     start=True, stop=False)
                    nc.tensor.matmul(out=pD[:], lhsT=w1r_s[:],
                                     rhs=xT_s[:, t * 128:(t + 1) * 128],
                                     start=False, stop=True)
                    h1t = h1tp.tile([64, 128], b16, tag="h1tp")
                    nc.scalar.activation(h1t[:], pD[:], Relu, bias=b1_s[:])
                    h1T.append(h1t)
                    # node-major table row write, scaled by dinv_src
                    pT = psT.tile([128, 64], b16, tag="psT",
                                  padded_shape=[128, 1024])
                    nc.tensor.transpose(out=pT[:], in_=h1t[:],
                                        identity=identb_s[:64, :64])
                    tb = tbp.tile([128, 64], b16, tag="tbp")
                    nc.vector.tensor_tensor(
                        out=tb[:], in0=pT[:],
                        in1=dinvT_s[:, t:t + 1].broadcast_to([128, 64]),
                        op=MULT)
                    nc.sync.dma_start(h1d_own[t * 128:(t + 1) * 128, :], tb[:])
                    if debug:
                        nc.sync.dma_start(
                            t_dbg_h1T[:, t * 128:(t + 1) * 128], h1t[:])
                        nc.sync.dma_start(
                            t_dbg_h1d[t * 128:(t + 1) * 128, :], tb[:])
            # ---------------- AllGather of the layer-1 table -----------------
            nc.gpsimd.collective_compute(
                "AllGather", mybir.AluOpType.bypass,
                replica_groups=[list(range(N_CORES))],
                ins=[h1d_own.opt()],
                outs=[h1d_full.opt()],
            )

            # ---------------- C2: layer-2 aggregation + output ---------------
            for g in range(NG):
                nch2 = G * 2 * c_tb2
                pC = psC.tile([128, 1024], f32, tag="psC")
                for bk in range(2):
                    nc.tensor.matmul(out=pC[:, bk * 512:(bk + 1) * 512],
                                     lhsT=zero_s[:], rhs=colr1_s[:, 0:512],
                                     start=True, stop=False)
                for b in range(NBLK):
                    off = (g * NBLK + b) * N2
                    m = mp.tile([128, N2], b16, tag="mp")
                    tab = h1d_full[b * BLK:(b + 1) * BLK, :]
                    done = 0
                    while done < N2:
                        step = min(SUBCALL, N2 - done)
                        nc.gpsimd.dma_gather(
                            out_ap=m[:, done:done + step].rearrange(
                                "p (c d) -> p c d", d=128),
                            in_ap=tab.rearrange("(p two) f -> p (two f)", two=2),
                            idxs_ap=idx2_s[:, (off + done) // 16:
                                           (off + done + step) // 16],
                            num_idxs=step,
                            num_idxs_reg=step,
                            elem_size=128,
                        )
                        done += step
                    jg = off // 128
                    oh = ohp.tile([128, nch2 * 128], b16, tag="ohp")
                    nc.vector.tensor_tensor(
                        out=oh[:].rearrange("p (c d) -> p c d", d=128),
                        in0=iota_s[:, 0:128].unsqueeze(1).broadcast_to(
                            [128, nch2, 128]),
                        in1=colr2_s[:, jg:jg + nch2].unsqueeze(2).broadcast_to(
                            [128, nch2, 128]),
                        op=ISEQ,
                    )
                    for ti in range(G):
                        last_of_bank = min(G - 1, (ti // 4) * 4 + 3)
                        for par in range(2):
                            for c in range(c_tb2):
                                j = (ti * 2 + par) * c_tb2 + c
                                nc.tensor.matmul(
                                    out=pC[:64, ti * 128:(ti + 1) * 128],
                                    lhsT=m[:, j * 128 + par * 64:
                                           j * 128 + par * 64 + 64],
                                    rhs=oh[:, j * 128:(j + 1) * 128],
                                    start=False,
                                    stop=(b == NBLK - 1 and par == 1
                                          and c == c_tb2 - 1
                                          and ti == last_of_bank),
                                )
                for ti in range(G):
                    t = g * G + ti
                    agg = aggp.tile([64, 128], b16, tag="aggp")
                    nc.vector.tensor_tensor(
                        out=agg[:], in0=pC[:64, ti * 128:(ti + 1) * 128],
                        in1=dinvb_s[:64, t * 128:(t + 1) * 128], op=MULT)
                    if debug:
                        nc.sync.dma_start(
                            t_dbg_agg2[:, t * 128:(t + 1) * 128], agg[:])
                    pO = psD.tile([64, 128], f32, tag="psD",
                                  padded_shape=[128, 512])
                    nc.tensor.matmul(out=pO[:], lhsT=w2i_s[:], rhs=agg[:],
                                     start=True, stop=False)
                    nc.tensor.matmul(out=pO[:], lhsT=w2r_s[:], rhs=h1T[t][:],
                                     start=False, stop=True)
                    ot = outp.tile([64, 128], f32, tag="outp")
                    nc.scalar.activation(ot[:], pO[:], Relu, bias=b2_s[:])
                    nc.sync.dma_start(t_out[:, t * 128:(t + 1) * 128], ot[:])

    nc.compile()
    return nc


# ----------------------------------------------------------------------------
# entry point
# ----------------------------------------------------------------------------

_LAST_RESULTS = None


def _in_maps(x, w1_init, w1_root, b1, w2_init, w2_root, b2, streams, dinv):
    x = np.asarray(x, np.float32)
    xtab = np.zeros((VPAD, 128), bf16)
    xtab[:N_NODES] = (dinv[:, None] * x).astype(bf16)
    iota = np.broadcast_to(np.arange(256, dtype=np.float32), (128, 256)).astype(bf16)
    identb = np.eye(128, dtype=np.float32).astype(bf16)
    w1i = np.asarray(w1_init, np.float32).astype(bf16)            # [128, 64]
    w1r = np.asarray(w1_root, np.float32).astype(bf16)            # [128, 64]
    w2i = np.zeros((64, 64), np.float32)
    w2i[:, :CLS] = np.asarray(w2_init)
    w2r = np.zeros((64, 64), np.float32)
    w2r[:, :CLS] = np.asarray(w2_root)
    b1c = np.asarray(b1, np.float32).reshape(64, 1).copy()
    b2c = np.zeros((64, 1), np.float32)
    b2c[:CLS, 0] = np.asarray(b2)

    c_tb1 = streams[0]["idx1"].shape[1] * 16 // (NT * NBLK * 128)
    c_tb2 = streams[0]["idx2"].shape[1] * 16 // (NT * NBLK * 2 * 128)
    lay, P16W = _pack_layout(c_tb1, c_tb2)

    def put(pack, name, arr):
        off, w = lay[name]
        arr = np.asarray(arr, dtype=np.float32).astype(bf16)
        pack[:arr.shape[0], off:off + arr.shape[1]] = arr

    maps = []
    for k in range(N_CORES):
        s = streams[k]
        perm = s["perm"]
        v = perm >= 0
        xT = np.zeros((128, SHARD_PAD), np.float32)
        xT[:, v] = x[perm[v]].T
        dinvb = np.broadcast_to(s["dinv_slot"], (128, SHARD_PAD))
        dinvT = s["dinv_slot"].reshape(NT, 128).T.copy()   # [128 slot, NT]
        pack16 = np.zeros((128, P16W), bf16)
        put(pack16, "xTown", xT)
        put(pack16, "dinvb", dinvb)
        put(pack16, "iota", np.asarray(iota))
        put(pack16, "identb", identb)
        put(pack16, "w1i", w1i)
        put(pack16, "w1r", w1r)
        put(pack16, "w2i", w2i)
        put(pack16, "w2r", w2r)
        pack16[:, lay["colr1"][0]:lay["colr1"][0] + lay["colr1"][1]] = s["colr1"]
        pack16[:, lay["colr2"][0]:lay["colr2"][0] + lay["colr2"][1]] = s["colr2"]
        packi = np.concatenate([s["idx1"], s["idx2"]], axis=1)
        packf = np.zeros((128, NT + 2), np.float32)
        packf[:, :NT] = dinvT
        packf[:64, NT] = b1c[:, 0]
        packf[:64, NT + 1] = b2c[:, 0]
        maps.append(dict(xtab=xtab, pack16=pack16, packi=packi, packf=packf))
    return maps


def kernel(x, edge_index, w1_init, w1_root, b1, w2_init, w2_root, b2, **kw):
    global _LAST_RESULTS
    from concourse.bass_utils import run_bass_kernel_spmd

    streams, c_tb1, c_tb2, dinv = _prep(np.asarray(edge_index))
    key = (c_tb1, c_tb2)
    if key not in _PROG_CACHE:
        _PROG_CACHE[key] = _build_program(c_tb1, c_tb2)
    nc = _PROG_CACHE[key]

    maps = _in_maps(x, w1_init, w1_root, b1, w2_init, w2_root, b2, streams,
                    dinv)
    res = run_bass_kernel_spmd(nc, maps, core_ids=list(range(N_CORES)))
    _LAST_RESULTS = res

    out = np.zeros((N_NODES, CLS), np.float32)
    for k in range(N_CORES):
        o = res.results[k]["outT"]          # [64, SHARD_PAD]
        perm = streams[k]["perm"]
        v = perm >= 0
        out[perm[v]] = o[:CLS, v].T
    return out
